# revision 26
# baseline (speedup 1.0000x reference)
"""Trainium2 Bass kernel for nn_DFDgraph (gnn_message_passing).

Pipeline per batch element (one NeuronCore each, 8 total):
  x (2048, 288) --rfft-mag--> (2048, 145) --minmax+l2--> xn
  h = LN(relu(cat[xn @ Wd0, te_norm] @ We0))            (2048, 64)
  adj = relu((h * w) @ h^T)                             (2048, 2048)
  out = top10_row_mask(adj) / (rowsum_kept + 1e-5)

The rfft is two matmuls against a host-precomputed DFT cos|sin matrix
(288 x 290, ortho-normalized), keeping everything fp32 (f32r loses
~1e-3 relative accuracy, which flips top-k selections). Top-10 per row:
DVE max8 -> match_replace(0) -> max8 gives the 10 largest values
exactly; kept = (adj >= v10) * adj via scalar_tensor_tensor on GpSimd,
final scale 1/(sum_top10 + 1e-5) on ACT.

Phase 1 is processed in groups of 4 row-tiles so the per-group stages
(DFT -> normalize -> MLP -> LN -> transpose) pipeline across groups.
"""

import numpy as np
from contextlib import ExitStack

import concourse.bass as bass
import concourse.mybir as mybir
from concourse import bacc
from concourse import tile
from concourse import masks
from concourse.bass_utils import run_bass_kernel_spmd

F32 = mybir.dt.float32
AX = mybir.AxisListType
OP = mybir.AluOpType
AF = mybir.ActivationFunctionType

B, N, T, H, EMB, TOPK = 8, 2048, 288, 64, 24, 10
F = T // 2 + 1          # 145
P = 128                 # rows per tile
NT = N // P             # 16 tiles
G = 4                   # tiles per pipeline group
KC = 96                 # DFT contraction chunk (3 x 96 = 288)
NCORES = 8

_CACHE = {}


def _build(sel_engine="gpsimd"):
    nc = bacc.Bacc("TRN2", target_bir_lowering=False, debug=False,
                   num_devices=NCORES)
    x_d = nc.declare_dram_parameter("x", [N, T], F32, isOutput=False)
    te_d = nc.declare_dram_parameter("t_emb", [N, EMB], F32, isOutput=False)
    cc_d = nc.declare_dram_parameter("ccos", [T, F], F32, isOutput=False)
    cs_d = nc.declare_dram_parameter("csin", [T, F], F32, isOutput=False)
    wd_d = nc.declare_dram_parameter("wd0", [F, H], F32, isOutput=False)
    we_d = nc.declare_dram_parameter("we0", [H + EMB, H], F32, isOutput=False)
    w_d = nc.declare_dram_parameter("w", [H, 1], F32, isOutput=False)
    out_d = nc.declare_dram_parameter("out", [N, N], F32, isOutput=True)

    with tile.TileContext(nc) as tc, ExitStack() as ctx:
        const = ctx.enter_context(tc.tile_pool(name="const", bufs=1))
        ident = const.tile([P, P], F32)
        masks.make_identity(nc, ident[:])
        ccs_sb = const.tile([KC, 3, 2 * F], F32)
        for c in range(3):
            nc.sync.dma_start(ccs_sb[:, c, 0:F], cc_d[c * KC:(c + 1) * KC, :])
            nc.sync.dma_start(ccs_sb[:, c, F:2 * F], cs_d[c * KC:(c + 1) * KC, :])
        wd_a = const.tile([P, H], F32)
        wd_b = const.tile([F - P, H], F32)
        nc.sync.dma_start(wd_a[:], wd_d[0:P, :])
        nc.sync.dma_start(wd_b[:], wd_d[P:F, :])
        we_sb = const.tile([H + EMB, H], F32)
        nc.sync.dma_start(we_sb[:], we_d[:])
        w_sb = const.tile([H, 1], F32)
        nc.sync.dma_start(w_sb[:], w_d[:])

        # persistent phase-1 results
        p1 = ctx.enter_context(tc.tile_pool(name="p1", bufs=1))
        hT_sb = p1.tile([H, N], F32)
        hTw_sb = p1.tile([H, N], F32)
        # [P, NT] stats, persistent
        st = ctx.enter_context(tc.tile_pool(name="stats", bufs=1))
        mx_s = st.tile([P, NT], F32)
        mn_s = st.tile([P, NT], F32)
        rd_s = st.tile([P, NT], F32)
        ssx_s = st.tile([P, NT], F32)
        rnx_s = st.tile([P, NT], F32)
        mxt_s = st.tile([P, NT], F32)
        mnt_s = st.tile([P, NT], F32)
        rdt_s = st.tile([P, NT], F32)
        sst_s = st.tile([P, NT], F32)
        rnt_s = st.tile([P, NT], F32)
        sums_s = st.tile([P, NT], F32)
        mean_s = st.tile([P, NT], F32)
        ssh_s = st.tile([P, NT], F32)
        rstd_s = st.tile([P, NT], F32)
        mnr_s = st.tile([P, NT], F32)

        # group-cycled working buffers (bufs=2 -> group g+1 overlaps group g)
        p1ps = ExitStack()
        gp = p1ps.enter_context(tc.tile_pool(name="gp", bufs=2))
        ps_a = p1ps.enter_context(tc.tile_pool(name="ps_a", bufs=2, space="PSUM"))
        ps_b = p1ps.enter_context(tc.tile_pool(name="ps_b", bufs=2, space="PSUM"))

        for g in range(NT // G):
            t0 = g * G
            sl = slice(t0, t0 + G)
            # ---- stage A: load x/te, transpose, DFT, squares ----
            re2 = gp.tile([P, G, F], F32, tag="re2")
            im2 = gp.tile([P, G, F], F32, tag="im2")
            te_g = gp.tile([P, G, EMB], F32, tag="te")
            for j in range(G):
                t = t0 + j
                x_t = gp.tile([P, T], F32, tag="x")
                nc.sync.dma_start(x_t[:], x_d[t * P:(t + 1) * P, :])
                nc.sync.dma_start(te_g[:, j, :], te_d[t * P:(t + 1) * P, :])
                xT = gp.tile([KC, 3, P], F32, tag="xT")
                for c in range(3):
                    ps = ps_a.tile([KC, P], F32, tag="xT_ps")
                    nc.tensor.transpose(ps[:], x_t[:, c * KC:(c + 1) * KC], ident[:])
                    nc.vector.tensor_copy(xT[:, c, :], ps[:])
                ri_ps = ps_a.tile([P, 2 * F], F32, tag="ri_ps")
                for c in range(3):
                    nc.tensor.matmul(ri_ps[:], lhsT=xT[:, c, :], rhs=ccs_sb[:, c, :],
                                     start=(c == 0), stop=(c == 2))
                nc.scalar.square(re2[:, j, :], ri_ps[:, 0:F])
                nc.scalar.square(im2[:, j, :], ri_ps[:, F:2 * F])

            # ---- stage B: mag, minmax, xn, l2 sums (batched per group) ----
            mag = gp.tile([P, G, F], F32, tag="mag")
            nc.gpsimd.tensor_add(mag[:], re2[:], im2[:])
            nc.scalar.sqrt(mag[:], mag[:])
            nc.vector.tensor_reduce(mx_s[:, sl], mag[:], axis=AX.X, op=OP.max)
            nc.vector.tensor_reduce(mn_s[:, sl], mag[:], axis=AX.X, op=OP.min)
            nc.vector.scalar_tensor_tensor(rd_s[:, sl], mx_s[:, sl], 1.0, mn_s[:, sl],
                                           op0=OP.add, op1=OP.subtract)
            nc.vector.reciprocal(rd_s[:, sl], rd_s[:, sl])
            nc.vector.tensor_reduce(mxt_s[:, sl], te_g[:], axis=AX.X, op=OP.max)
            nc.vector.tensor_reduce(mnt_s[:, sl], te_g[:], axis=AX.X, op=OP.min)
            nc.vector.scalar_tensor_tensor(rdt_s[:, sl], mxt_s[:, sl], 1.0, mnt_s[:, sl],
                                           op0=OP.add, op1=OP.subtract)
            nc.vector.reciprocal(rdt_s[:, sl], rdt_s[:, sl])
            xn_g = gp.tile([P, G, F], F32, tag="xn")
            ten_g = gp.tile([P, G, EMB], F32, tag="ten")
            for j in range(G):
                t = t0 + j
                nc.gpsimd.tensor_scalar(xn_g[:, j, :], mag[:, j, :],
                                        scalar1=mn_s[:, t:t + 1], scalar2=rd_s[:, t:t + 1],
                                        op0=OP.subtract, op1=OP.mult)
                scr = gp.tile([P, F], F32, tag="scrF")
                nc.scalar.activation(scr[:], xn_g[:, j, :], AF.Square,
                                     accum_out=ssx_s[:, t:t + 1])
                nc.gpsimd.tensor_scalar(ten_g[:, j, :], te_g[:, j, :],
                                        scalar1=mnt_s[:, t:t + 1], scalar2=rdt_s[:, t:t + 1],
                                        op0=OP.subtract, op1=OP.mult)
                scr2 = gp.tile([P, EMB], F32, tag="scrE")
                nc.scalar.activation(scr2[:], ten_g[:, j, :], AF.Square,
                                     accum_out=sst_s[:, t:t + 1])
            nc.scalar.sqrt(ssx_s[:, sl], ssx_s[:, sl])
            nc.vector.reciprocal(rnx_s[:, sl], ssx_s[:, sl])
            nc.scalar.sqrt(sst_s[:, sl], sst_s[:, sl])
            nc.vector.reciprocal(rnt_s[:, sl], sst_s[:, sl])

            # ---- stage C: q = xn @ Wd0, cat, h = relu(cat @ We0) ----
            hr_g = gp.tile([P, G, H], F32, tag="hr")
            for j in range(G):
                t = t0 + j
                pa = ps_b.tile([P, P], F32, tag="tp_ps")
                nc.tensor.transpose(pa[:], xn_g[:, j, 0:P], ident[:])
                pb = ps_b.tile([F - P, P], F32, tag="tp_ps")
                nc.tensor.transpose(pb[:], xn_g[:, j, P:F], ident[:])
                xnT_a = gp.tile([P, P], F32, tag="xnT_a")
                xnT_b = gp.tile([F - P, P], F32, tag="xnT_b")
                nc.vector.tensor_copy(xnT_a[:], pa[:])
                nc.vector.tensor_copy(xnT_b[:], pb[:])
                q_ps = ps_b.tile([P, H], F32, tag="mm_ps")
                nc.tensor.matmul(q_ps[:], lhsT=xnT_a[:], rhs=wd_a[:], start=True, stop=False)
                nc.tensor.matmul(q_ps[:], lhsT=xnT_b[:], rhs=wd_b[:], start=False, stop=True)
                cat_t = gp.tile([P, H + EMB], F32, tag="cat")
                nc.scalar.activation(cat_t[:, 0:H], q_ps[:], AF.Copy,
                                     scale=rnx_s[:, t:t + 1])
                nc.gpsimd.tensor_scalar_mul(cat_t[:, H:H + EMB], ten_g[:, j, :],
                                            rnt_s[:, t:t + 1])
                pc = ps_b.tile([H + EMB, P], F32, tag="mm_ps")
                nc.tensor.transpose(pc[:], cat_t[:], ident[:])
                catT = gp.tile([H + EMB, P], F32, tag="catT")
                nc.vector.tensor_copy(catT[:], pc[:])
                h_ps = ps_b.tile([P, H], F32, tag="mm_ps")
                nc.tensor.matmul(h_ps[:], lhsT=catT[:], rhs=we_sb[:], start=True, stop=True)
                nc.scalar.activation(hr_g[:, j, :], h_ps[:], AF.Relu,
                                     accum_out=sums_s[:, t:t + 1])

            # ---- stage D: LN + transpose into hT / hTw ----
            nc.vector.tensor_scalar_mul(mean_s[:, sl], sums_s[:, sl], -1.0 / H)
            for j in range(G):
                t = t0 + j
                scr = gp.tile([P, H], F32, tag="scrH")
                nc.scalar.activation(scr[:], hr_g[:, j, :], AF.Square,
                                     bias=mean_s[:, t:t + 1],
                                     accum_out=ssh_s[:, t:t + 1])
            nc.vector.tensor_scalar(ssh_s[:, sl], ssh_s[:, sl], scalar1=1.0 / H,
                                    scalar2=1e-8, op0=OP.mult, op1=OP.add)
            nc.scalar.sqrt(ssh_s[:, sl], ssh_s[:, sl])
            nc.vector.reciprocal(rstd_s[:, sl], ssh_s[:, sl])
            nc.vector.tensor_mul(mnr_s[:, sl], mean_s[:, sl], rstd_s[:, sl])
            for j in range(G):
                t = t0 + j
                h_t = gp.tile([P, H], F32, tag="h_t")
                nc.scalar.activation(h_t[:], hr_g[:, j, :], AF.Identity,
                                     scale=rstd_s[:, t:t + 1],
                                     bias=mnr_s[:, t:t + 1])
                hT_ps = ps_b.tile([H, P], F32, tag="mm_ps")
                nc.tensor.transpose(hT_ps[:], h_t[:], ident[:])
                nc.vector.tensor_copy(hT_sb[:, t * P:(t + 1) * P], hT_ps[:])
                nc.vector.tensor_scalar_mul(hTw_sb[:, t * P:(t + 1) * P], hT_ps[:],
                                            w_sb[:, 0:1])

        p1ps.close()

        # ---- phase 2: adjacency + top-k + normalize ----
        with tc.tile_pool(name="p2_sb", bufs=5) as p2_sb, \
             tc.tile_pool(name="p2_zap", bufs=3) as p2_zap, \
             tc.tile_pool(name="p2_sm", bufs=6) as p2_sm, \
             tc.tile_pool(name="p2_ps", bufs=4, space="PSUM") as p2_ps:
            for m in range(NT):
                adj_sb = p2_sb.tile([P, N], F32, tag="adj_sb")
                for half in range(2):
                    adj_ps = p2_ps.tile([P, N // 2], F32, tag="adj_ps")
                    for n in range(2):
                        c0 = half * 1024 + n * 512
                        nc.tensor.matmul(adj_ps[:, n * 512:(n + 1) * 512],
                                         lhsT=hTw_sb[:, m * P:(m + 1) * P],
                                         rhs=hT_sb[:, c0:c0 + 512],
                                         start=True, stop=True)
                    nc.scalar.activation(adj_sb[:, half * 1024:(half + 1) * 1024],
                                         adj_ps[:], AF.Relu)
                mx16 = p2_sm.tile([P, 16], F32, tag="mx16")
                nc.vector.max(mx16[:, 0:8], adj_sb[:])
                zap = p2_zap.tile([P, N], F32, tag="zap")
                nc.vector.match_replace(zap[:], in_to_replace=mx16[:, 0:8],
                                        in_values=adj_sb[:], imm_value=0.0)
                nc.vector.max(mx16[:, 8:16], zap[:])
                den = p2_sm.tile([P, 1], F32, tag="den")
                nc.vector.tensor_reduce(den[:], mx16[:, 0:TOPK], axis=AX.X, op=OP.add)
                r = p2_sm.tile([P, 1], F32, tag="r")
                nc.vector.tensor_scalar_add(den[:], den[:], 1e-5)
                nc.vector.reciprocal(r[:], den[:])
                sel = p2_sb.tile([P, N], F32, tag="sel")
                if sel_engine == "gpsimd":
                    # mask = (adj >= v10) on Pool (depends only on v10), then
                    # unscaled keep on Pool; r-scale on ACT (r off critical path)
                    nc.gpsimd.tensor_scalar(sel[:], adj_sb[:],
                                            scalar1=mx16[:, TOPK - 1:TOPK],
                                            scalar2=None, op0=OP.is_ge)
                    nc.gpsimd.tensor_tensor(sel[:], sel[:], adj_sb[:], op=OP.mult)
                else:
                    nc.vector.scalar_tensor_tensor(sel[:], adj_sb[:],
                                                   mx16[:, TOPK - 1:TOPK], adj_sb[:],
                                                   op0=OP.is_ge, op1=OP.mult)
                outt = p2_sb.tile([P, N], F32, tag="outt")
                nc.scalar.activation(outt[:], sel[:], AF.Copy, scale=r[:, 0:1])
                nc.sync.dma_start(out_d[m * P:(m + 1) * P, :], outt[:])

    nc.compile()
    return nc


def _dft_mats():
    tt = np.arange(T)[:, None].astype(np.float64)
    kk = np.arange(F)[None, :].astype(np.float64)
    ang = 2.0 * np.pi * tt * kk / T
    s = 1.0 / np.sqrt(T)
    return (np.cos(ang) * s).astype(np.float32), (np.sin(ang) * s).astype(np.float32)


def kernel(x, t_emb, Wd0, We0, W):
    if "nc" not in _CACHE:
        _CACHE["nc"] = _build()
    nc = _CACHE["nc"]
    cc, cs = _dft_mats()
    base = {
        "ccos": cc, "csin": cs,
        "wd0": np.ascontiguousarray(Wd0, np.float32),
        "we0": np.ascontiguousarray(We0, np.float32),
        "w": np.ascontiguousarray(W, np.float32),
    }
    in_maps = [
        {**base,
         "x": np.ascontiguousarray(x[i], np.float32),
         "t_emb": np.ascontiguousarray(t_emb[i], np.float32)}
        for i in range(NCORES)
    ]
    res = run_bass_kernel_spmd(nc, in_maps, list(range(NCORES)))
    return np.stack([res.results[i]["out"] for i in range(NCORES)], axis=0)


# revision 29
# speedup vs baseline: 15437.8499x; 15437.8499x over previous
"""Trainium2 Bass kernel for nn_DFDgraph (gnn_message_passing).

Pipeline per batch element (one NeuronCore each, 8 total):
  x (2048, 288) --rfft-mag--> (2048, 145) --minmax+l2--> xn
  h = LN(relu(cat[xn @ Wd0, te_norm] @ We0))            (2048, 64)
  adj = relu((h * w) @ h^T)                             (2048, 2048)
  out = top10_row_mask(adj) / (rowsum_kept + 1e-5)

The rfft is two matmuls against a host-precomputed DFT cos|sin matrix
(288 x 290, ortho-normalized), keeping everything fp32 (f32r loses
~1e-3 relative accuracy, which flips top-k selections). Top-10 per row:
DVE max8 -> match_replace(0) -> max8 gives the 10 largest values
exactly; kept = (adj >= v10) * adj via scalar_tensor_tensor on GpSimd,
final scale 1/(sum_top10 + 1e-5) on ACT.

Phase 1 is processed in groups of 4 row-tiles so the per-group stages
(DFT -> normalize -> MLP -> LN -> transpose) pipeline across groups.
"""

import numpy as np
from contextlib import ExitStack

import concourse.bass as bass
import concourse.mybir as mybir
from concourse import bacc
from concourse import tile
from concourse import masks
from concourse.bass_utils import run_bass_kernel_spmd

F32 = mybir.dt.float32
AX = mybir.AxisListType
OP = mybir.AluOpType
AF = mybir.ActivationFunctionType

B, N, T, H, EMB, TOPK = 8, 2048, 288, 64, 24, 10
F = T // 2 + 1          # 145
P = 128                 # rows per tile
NT = N // P             # 16 tiles
G = 4                   # tiles per pipeline group
KC = 96                 # DFT contraction chunk (3 x 96 = 288)
NCORES = 8

_CACHE = {}


def _build(sel_engine="gpsimd"):
    nc = bacc.Bacc("TRN2", target_bir_lowering=False, debug=False,
                   num_devices=NCORES)
    x_d = nc.declare_dram_parameter("x", [N, T], F32, isOutput=False)
    te_d = nc.declare_dram_parameter("t_emb", [N, EMB], F32, isOutput=False)
    cc_d = nc.declare_dram_parameter("ccos", [T, F], F32, isOutput=False)
    cs_d = nc.declare_dram_parameter("csin", [T, F], F32, isOutput=False)
    wd_d = nc.declare_dram_parameter("wd0", [F, H], F32, isOutput=False)
    we_d = nc.declare_dram_parameter("we0", [H + EMB, H], F32, isOutput=False)
    w_d = nc.declare_dram_parameter("w", [H, 1], F32, isOutput=False)
    out_d = nc.declare_dram_parameter("out", [N, N], F32, isOutput=True)

    with tile.TileContext(nc) as tc, ExitStack() as ctx:
        const = ctx.enter_context(tc.tile_pool(name="const", bufs=1))
        ident = const.tile([P, P], F32)
        masks.make_identity(nc, ident[:])
        ccs_sb = const.tile([KC, 3, 2 * F], F32)
        for c in range(3):
            nc.sync.dma_start(ccs_sb[:, c, 0:F], cc_d[c * KC:(c + 1) * KC, :])
            nc.sync.dma_start(ccs_sb[:, c, F:2 * F], cs_d[c * KC:(c + 1) * KC, :])
        wd_a = const.tile([P, H], F32)
        wd_b = const.tile([F - P, H], F32)
        nc.sync.dma_start(wd_a[:], wd_d[0:P, :])
        nc.sync.dma_start(wd_b[:], wd_d[P:F, :])
        we_sb = const.tile([H + EMB, H], F32)
        nc.sync.dma_start(we_sb[:], we_d[:])
        w_sb = const.tile([H, 1], F32)
        nc.sync.dma_start(w_sb[:], w_d[:])

        # persistent phase-1 results
        p1 = ctx.enter_context(tc.tile_pool(name="p1", bufs=1))
        hT_sb = p1.tile([H, N], F32)
        hTw_sb = p1.tile([H, N], F32)
        # [P, NT] stats, persistent
        st = ctx.enter_context(tc.tile_pool(name="stats", bufs=1))
        mx_s = st.tile([P, NT], F32)
        mn_s = st.tile([P, NT], F32)
        rd_s = st.tile([P, NT], F32)
        ssx_s = st.tile([P, NT], F32)
        rnx_s = st.tile([P, NT], F32)
        mxt_s = st.tile([P, NT], F32)
        mnt_s = st.tile([P, NT], F32)
        rdt_s = st.tile([P, NT], F32)
        sst_s = st.tile([P, NT], F32)
        rnt_s = st.tile([P, NT], F32)
        sums_s = st.tile([P, NT], F32)
        mean_s = st.tile([P, NT], F32)
        ssh_s = st.tile([P, NT], F32)
        rstd_s = st.tile([P, NT], F32)
        mnr_s = st.tile([P, NT], F32)

        # group-cycled working buffers (bufs=2 -> group g+1 overlaps group g)
        p1ps = ExitStack()
        gp = p1ps.enter_context(tc.tile_pool(name="gp", bufs=2))
        ps_a = p1ps.enter_context(tc.tile_pool(name="ps_a", bufs=2, space="PSUM"))
        ps_b = p1ps.enter_context(tc.tile_pool(name="ps_b", bufs=2, space="PSUM"))

        for g in range(NT // G):
            t0 = g * G
            sl = slice(t0, t0 + G)
            # ---- stage A: load x/te, transpose, DFT, squares ----
            re2 = gp.tile([P, G, F], F32, tag="re2")
            im2 = gp.tile([P, G, F], F32, tag="im2")
            te_g = gp.tile([P, G, EMB], F32, tag="te")
            for j in range(G):
                t = t0 + j
                x_t = gp.tile([P, T], F32, tag="x")
                nc.sync.dma_start(x_t[:], x_d[t * P:(t + 1) * P, :])
                nc.sync.dma_start(te_g[:, j, :], te_d[t * P:(t + 1) * P, :])
                xT = gp.tile([KC, 3, P], F32, tag="xT")
                for c in range(3):
                    ps = ps_a.tile([KC, P], F32, tag="xT_ps")
                    nc.tensor.transpose(ps[:], x_t[:, c * KC:(c + 1) * KC], ident[:])
                    nc.vector.tensor_copy(xT[:, c, :], ps[:])
                ri_ps = ps_a.tile([P, 2 * F], F32, tag="ri_ps")
                for c in range(3):
                    nc.tensor.matmul(ri_ps[:], lhsT=xT[:, c, :], rhs=ccs_sb[:, c, :],
                                     start=(c == 0), stop=(c == 2))
                nc.scalar.square(re2[:, j, :], ri_ps[:, 0:F])
                nc.scalar.square(im2[:, j, :], ri_ps[:, F:2 * F])

            # ---- stage B: mag, minmax, xn, l2 sums (batched per group) ----
            mag = gp.tile([P, G, F], F32, tag="mag")
            nc.gpsimd.tensor_add(mag[:], re2[:], im2[:])
            nc.scalar.sqrt(mag[:], mag[:])
            nc.vector.tensor_reduce(mx_s[:, sl], mag[:], axis=AX.X, op=OP.max)
            nc.vector.tensor_reduce(mn_s[:, sl], mag[:], axis=AX.X, op=OP.min)
            nc.vector.scalar_tensor_tensor(rd_s[:, sl], mx_s[:, sl], 1.0, mn_s[:, sl],
                                           op0=OP.add, op1=OP.subtract)
            nc.vector.reciprocal(rd_s[:, sl], rd_s[:, sl])
            nc.vector.tensor_reduce(mxt_s[:, sl], te_g[:], axis=AX.X, op=OP.max)
            nc.vector.tensor_reduce(mnt_s[:, sl], te_g[:], axis=AX.X, op=OP.min)
            nc.vector.scalar_tensor_tensor(rdt_s[:, sl], mxt_s[:, sl], 1.0, mnt_s[:, sl],
                                           op0=OP.add, op1=OP.subtract)
            nc.vector.reciprocal(rdt_s[:, sl], rdt_s[:, sl])
            xn_g = gp.tile([P, G, F], F32, tag="xn")
            ten_g = gp.tile([P, G, EMB], F32, tag="ten")
            for j in range(G):
                t = t0 + j
                nc.gpsimd.tensor_scalar(xn_g[:, j, :], mag[:, j, :],
                                        scalar1=mn_s[:, t:t + 1], scalar2=rd_s[:, t:t + 1],
                                        op0=OP.subtract, op1=OP.mult)
                scr = gp.tile([P, F], F32, tag="scrF")
                nc.scalar.activation(scr[:], xn_g[:, j, :], AF.Square,
                                     accum_out=ssx_s[:, t:t + 1])
                nc.gpsimd.tensor_scalar(ten_g[:, j, :], te_g[:, j, :],
                                        scalar1=mnt_s[:, t:t + 1], scalar2=rdt_s[:, t:t + 1],
                                        op0=OP.subtract, op1=OP.mult)
                scr2 = gp.tile([P, EMB], F32, tag="scrE")
                nc.scalar.activation(scr2[:], ten_g[:, j, :], AF.Square,
                                     accum_out=sst_s[:, t:t + 1])
            nc.scalar.sqrt(ssx_s[:, sl], ssx_s[:, sl])
            nc.vector.reciprocal(rnx_s[:, sl], ssx_s[:, sl])
            nc.scalar.sqrt(sst_s[:, sl], sst_s[:, sl])
            nc.vector.reciprocal(rnt_s[:, sl], sst_s[:, sl])

            # ---- stage C: q = xn @ Wd0, cat, h = relu(cat @ We0) ----
            hr_g = gp.tile([P, G, H], F32, tag="hr")
            for j in range(G):
                t = t0 + j
                pa = ps_b.tile([P, P], F32, tag="tp_ps")
                nc.tensor.transpose(pa[:], xn_g[:, j, 0:P], ident[:])
                pb = ps_b.tile([F - P, P], F32, tag="tp_ps")
                nc.tensor.transpose(pb[:], xn_g[:, j, P:F], ident[:])
                xnT_a = gp.tile([P, P], F32, tag="xnT_a")
                xnT_b = gp.tile([F - P, P], F32, tag="xnT_b")
                nc.vector.tensor_copy(xnT_a[:], pa[:])
                nc.vector.tensor_copy(xnT_b[:], pb[:])
                q_ps = ps_b.tile([P, H], F32, tag="mm_ps")
                nc.tensor.matmul(q_ps[:], lhsT=xnT_a[:], rhs=wd_a[:], start=True, stop=False)
                nc.tensor.matmul(q_ps[:], lhsT=xnT_b[:], rhs=wd_b[:], start=False, stop=True)
                cat_t = gp.tile([P, H + EMB], F32, tag="cat")
                nc.scalar.activation(cat_t[:, 0:H], q_ps[:], AF.Copy,
                                     scale=rnx_s[:, t:t + 1])
                nc.gpsimd.tensor_scalar_mul(cat_t[:, H:H + EMB], ten_g[:, j, :],
                                            rnt_s[:, t:t + 1])
                pc = ps_b.tile([H + EMB, P], F32, tag="mm_ps")
                nc.tensor.transpose(pc[:], cat_t[:], ident[:])
                catT = gp.tile([H + EMB, P], F32, tag="catT")
                nc.vector.tensor_copy(catT[:], pc[:])
                h_ps = ps_b.tile([P, H], F32, tag="mm_ps")
                nc.tensor.matmul(h_ps[:], lhsT=catT[:], rhs=we_sb[:], start=True, stop=True)
                nc.scalar.activation(hr_g[:, j, :], h_ps[:], AF.Relu,
                                     accum_out=sums_s[:, t:t + 1])

            # ---- stage D: LN + transpose into hT / hTw ----
            nc.vector.tensor_scalar_mul(mean_s[:, sl], sums_s[:, sl], -1.0 / H)
            for j in range(G):
                t = t0 + j
                scr = gp.tile([P, H], F32, tag="scrH")
                nc.scalar.activation(scr[:], hr_g[:, j, :], AF.Square,
                                     bias=mean_s[:, t:t + 1],
                                     accum_out=ssh_s[:, t:t + 1])
            nc.vector.tensor_scalar(ssh_s[:, sl], ssh_s[:, sl], scalar1=1.0 / H,
                                    scalar2=1e-8, op0=OP.mult, op1=OP.add)
            nc.scalar.sqrt(ssh_s[:, sl], ssh_s[:, sl])
            nc.vector.reciprocal(rstd_s[:, sl], ssh_s[:, sl])
            nc.vector.tensor_mul(mnr_s[:, sl], mean_s[:, sl], rstd_s[:, sl])
            for j in range(G):
                t = t0 + j
                h_t = gp.tile([P, H], F32, tag="h_t")
                nc.scalar.activation(h_t[:], hr_g[:, j, :], AF.Identity,
                                     scale=rstd_s[:, t:t + 1],
                                     bias=mnr_s[:, t:t + 1])
                hT_ps = ps_b.tile([H, P], F32, tag="mm_ps")
                nc.tensor.transpose(hT_ps[:], h_t[:], ident[:])
                nc.vector.tensor_copy(hT_sb[:, t * P:(t + 1) * P], hT_ps[:])
                nc.vector.tensor_scalar_mul(hTw_sb[:, t * P:(t + 1) * P], hT_ps[:],
                                            w_sb[:, 0:1])

        p1ps.close()

        # ---- phase 2: adjacency + top-k + normalize ----
        with tc.tile_pool(name="p2_sb", bufs=5) as p2_sb, \
             tc.tile_pool(name="p2_zap", bufs=3) as p2_zap, \
             tc.tile_pool(name="p2_sm", bufs=6) as p2_sm, \
             tc.tile_pool(name="p2_ps", bufs=4, space="PSUM") as p2_ps:
            for m in range(NT):
                adj_sb = p2_sb.tile([P, N], F32, tag="adj_sb")
                for half in range(2):
                    adj_ps = p2_ps.tile([P, N // 2], F32, tag="adj_ps")
                    for n in range(2):
                        c0 = half * 1024 + n * 512
                        nc.tensor.matmul(adj_ps[:, n * 512:(n + 1) * 512],
                                         lhsT=hTw_sb[:, m * P:(m + 1) * P],
                                         rhs=hT_sb[:, c0:c0 + 512],
                                         start=True, stop=True)
                    nc.scalar.activation(adj_sb[:, half * 1024:(half + 1) * 1024],
                                         adj_ps[:], AF.Relu)
                mx16 = p2_sm.tile([P, 16], F32, tag="mx16")
                nc.vector.max(mx16[:, 0:8], adj_sb[:])
                zap = p2_zap.tile([P, N], F32, tag="zap")
                nc.vector.match_replace(zap[:], in_to_replace=mx16[:, 0:8],
                                        in_values=adj_sb[:], imm_value=0.0)
                nc.vector.max(mx16[:, 8:16], zap[:])
                den = p2_sm.tile([P, 1], F32, tag="den")
                nc.vector.tensor_reduce(den[:], mx16[:, 0:TOPK], axis=AX.X, op=OP.add)
                r = p2_sm.tile([P, 1], F32, tag="r")
                nc.vector.tensor_scalar_add(den[:], den[:], 1e-5)
                nc.vector.reciprocal(r[:], den[:])
                sel = p2_sb.tile([P, N], F32, tag="sel")
                if sel_engine == "gpsimd":
                    # mask = (adj >= v10) on Pool (depends only on v10), then
                    # unscaled keep on Pool; r-scale on ACT (r off critical path)
                    nc.gpsimd.tensor_scalar(sel[:], adj_sb[:],
                                            scalar1=mx16[:, TOPK - 1:TOPK],
                                            scalar2=None, op0=OP.is_ge)
                    nc.gpsimd.tensor_tensor(sel[:], sel[:], adj_sb[:], op=OP.mult)
                else:
                    nc.vector.scalar_tensor_tensor(sel[:], adj_sb[:],
                                                   mx16[:, TOPK - 1:TOPK], adj_sb[:],
                                                   op0=OP.is_ge, op1=OP.mult)
                outt = p2_sb.tile([P, N], F32, tag="outt")
                nc.scalar.activation(outt[:], sel[:], AF.Copy, scale=r[:, 0:1])
                nc.sync.dma_start(out_d[m * P:(m + 1) * P, :], outt[:])

    nc.compile()
    return nc


def _dft_mats():
    tt = np.arange(T)[:, None].astype(np.float64)
    kk = np.arange(F)[None, :].astype(np.float64)
    ang = 2.0 * np.pi * tt * kk / T
    s = 1.0 / np.sqrt(T)
    return (np.cos(ang) * s).astype(np.float32), (np.sin(ang) * s).astype(np.float32)


def kernel(x, t_emb, Wd0, We0, W):
    if "nc" not in _CACHE:
        _CACHE["nc"] = _build()
    nc = _CACHE["nc"]
    cc, cs = _dft_mats()
    base = {
        "ccos": cc, "csin": cs,
        "wd0": np.ascontiguousarray(Wd0, np.float32),
        "we0": np.ascontiguousarray(We0, np.float32),
        "w": np.ascontiguousarray(W, np.float32),
    }
    in_maps = [
        {**base,
         "x": np.ascontiguousarray(x[i], np.float32),
         "t_emb": np.ascontiguousarray(t_emb[i], np.float32)}
        for i in range(NCORES)
    ]
    res = run_bass_kernel_spmd(nc, in_maps, list(range(NCORES)))
    return np.stack([res.results[i]["out"] for i in range(NCORES)], axis=0)
